# revision 8
# baseline (speedup 1.0000x reference)
"""Trainium2 Bass kernel for nn_DAttention:
out[b,c,d,h,w] = x[b,c,d,h,w] * mean_{c,h,w}(x[b,:,d,:,:]).

Sharding: pure data parallel over batch B=8 -> one batch per NeuronCore.
Numerics: HBM I/O in bf16 (host converts f32->bf16 in, bf16->f32 out);
the mean is accumulated in fp32, so element error is ~2 bf16 roundings
(~0.4%) -- far inside the 2e-2 gate. 32 MiB in + 32 MiB out per core.

DMA load-balancing: SDMA engine k serves SBUF partitions 8k..8k+7
(measured: a [120, F] tile's traffic lands on engines 0-14 only).
Engine 15 runs ~15% slower than the rest under load (fat-tailed packet
durations, HW arbitration) and is the critical path of a uniform
[128, F] layout. So each 2^19-element d-slice is dealt (on the host)
into rows of 4128 elements on partitions 0-119 and rows of 3616
elements on partitions 120-127 (120*4128 + 8*3616 = 2^19): engine 15
carries 12% fewer bytes, absorbing its deficit. Both row widths are
64-byte aligned in DRAM (packet efficiency); the [120:128, 3616:4128]
pad strip of each pool buffer is zeroed once so full-rectangle
reductions stay exact (loads never overwrite the pad; the multiply
writes scaled zeros into the out tile's pad which is never stored).

Per-slice schedule (balanced so the post-load tail stays close to
store-DMA-paced):
  ACT: activation-Copy (dead PSUM scratch) with accum_out -> fp32
       column sums of xt[:, :A]
  DVE: tensor_reduce(add) -> fp32 column sums of xt[:, A:]
  PE : two accumulated fp32 matmuls against a constant 128x128 matrix
       of 1/2^19 -> total sum broadcast to [128,1] PSUM
  ACT: tiny copy mean PSUM->SBUF
  DVE: one tensor_scalar multiply (bf16 4 elem/cyc) over the full tile
  DMA: main+tail loads and the tiny tail store on the SP HWDGE ring,
       main store on the ACT ring; loads issued LOOKAHEAD slices early
"""
import numpy as np
import ml_dtypes

import concourse.bacc as bacc
import concourse.tile as tile
import concourse.mybir as mybir
from concourse.bass_utils import run_bass_kernel_spmd

BF16 = ml_dtypes.bfloat16

B, C, D, H, W = 8, 32, 32, 128, 128
N = C * H * W           # 524288 = 2**19 elements per (b, d) slice
RECIP = 1.0 / N         # exact in fp32
FO = 4128               # row width, partitions 0-119 (8256 B, 64B-aligned)
FT = 3616               # row width, partitions 120-127 (7232 B, aligned)
PM = 120
LM = PM * FO            # 495360 elems dealt to partitions 0-119
assert PM * FO + 8 * FT == N
A_SPLIT = 2670          # ACT reduces xt[:, :A], DVE reduces xt[:, A:]
LOOKAHEAD = 3
XIN_BUFS = 5

_NC = None


def _build_nc(xin_bufs=XIN_BUFS, out_bufs=3):
    nc = bacc.Bacc("TRN2", target_bir_lowering=False, debug=False)
    xa = nc.dram_tensor("xa", [D, PM, FO], mybir.dt.bfloat16, kind="ExternalInput")
    xb = nc.dram_tensor("xb", [D, 8, FT], mybir.dt.bfloat16, kind="ExternalInput")
    oa = nc.dram_tensor("oa", [D, PM, FO], mybir.dt.bfloat16, kind="ExternalOutput")
    ob = nc.dram_tensor("ob", [D, 8, FT], mybir.dt.bfloat16, kind="ExternalOutput")
    with tile.TileContext(nc) as tc:
        with (
            tc.tile_pool(name="xin", bufs=xin_bufs) as xpool,
            tc.tile_pool(name="oout", bufs=out_bufs) as opool,
            tc.tile_pool(name="small", bufs=6) as spool,
            tc.tile_pool(name="psum", bufs=2, space="PSUM") as ppool,
            tc.tile_pool(name="psc", bufs=1, space="PSUM") as scpool,
            tc.tile_pool(name="const", bufs=1) as cpool,
        ):
            recip = cpool.tile([128, 128], mybir.dt.float32)
            nc.gpsimd.memset(recip[:], RECIP)

            xts = {}
            n_alloc = 0

            def issue_loads(d):
                nonlocal n_alloc
                xt = xpool.tile([128, FO], mybir.dt.bfloat16, tag="xt")
                if n_alloc < xin_bufs:
                    # zero the pad strip once per physical buffer (engine
                    # ops need partition start % 32 == 0; rows 96-119 get
                    # overwritten by the load right after)
                    nc.vector.memset(xt[96:, FT:], 0.0)
                n_alloc += 1
                nc.sync.dma_start(xt[:PM, :], xa[d])
                nc.sync.dma_start(xt[PM:, :FT], xb[d])
                xts[d] = xt

            for d in range(LOOKAHEAD):
                issue_loads(d)
            for d in range(D):
                xt = xts.pop(d)
                csa = spool.tile([128, 1], mybir.dt.float32, tag="csa")
                csb = spool.tile([128, 1], mybir.dt.float32, tag="csb")
                scrA = scpool.tile([128, A_SPLIT], mybir.dt.float32, tag="scA")
                nc.scalar.activation(
                    scrA[:], xt[:, :A_SPLIT],
                    mybir.ActivationFunctionType.Copy, accum_out=csa[:],
                )
                nc.vector.tensor_reduce(
                    csb[:], xt[:, A_SPLIT:],
                    mybir.AxisListType.X, mybir.AluOpType.add,
                )
                dv = ppool.tile([128, 1], mybir.dt.float32, tag="dv")
                nc.tensor.matmul(dv[:], recip[:], csa[:], start=True, stop=False)
                nc.tensor.matmul(dv[:], recip[:], csb[:], start=False, stop=True)
                dvs = spool.tile([128, 1], mybir.dt.float32, tag="dvs")
                nc.scalar.copy(dvs[:], dv[:])
                ot = opool.tile([128, FO], mybir.dt.bfloat16, tag="ot")
                nc.vector.tensor_scalar_mul(ot[:], xt[:], dvs[:])
                if d + LOOKAHEAD < D:
                    issue_loads(d + LOOKAHEAD)
                nc.scalar.dma_start(oa[d], ot[:PM, :])
                nc.sync.dma_start(ob[d], ot[PM:, :FT])
    nc.compile()
    return nc


def _get_nc():
    global _NC
    if _NC is None:
        _NC = _build_nc()
    return _NC


def _deal_in(x_core: np.ndarray):
    """[C,D,H,W] f32 -> (xa [D,PM,FO], xb [D,8,FT]) bf16."""
    xd = np.ascontiguousarray(x_core.astype(BF16).transpose(1, 0, 2, 3)).reshape(D, N)
    xa = np.ascontiguousarray(xd[:, :LM]).reshape(D, PM, FO)
    xbt = np.ascontiguousarray(xd[:, LM:]).reshape(D, 8, FT)
    return xa, xbt


def _deal_out(oa_core: np.ndarray, ob_core: np.ndarray):
    """(oa [D,PM,FO], ob [D,8,FT]) bf16 -> [C,D,H,W] f32."""
    od = np.empty((D, N), BF16)
    od[:, :LM] = oa_core.reshape(D, LM)
    od[:, LM:] = ob_core.reshape(D, 8 * FT)
    return od.reshape(D, C, H, W).transpose(1, 0, 2, 3).astype(np.float32)


def run(x: np.ndarray, trace: bool = False, tmpdir: str | None = None):
    """Run on 8 NeuronCores; returns (out, BassKernelResults)."""
    x = np.asarray(x)
    assert x.shape == (B, C, D, H, W), x.shape
    nc = _get_nc()
    in_maps = []
    for b in range(B):
        xa, xbt = _deal_in(x[b])
        in_maps.append({"xa": xa, "xb": xbt})
    res = run_bass_kernel_spmd(
        nc, in_maps, core_ids=list(range(B)), trace=trace, tmpdir=tmpdir
    )
    out = np.stack([_deal_out(r["oa"], r["ob"]) for r in res.results])
    return out, res


def kernel(x: np.ndarray) -> np.ndarray:
    out, _ = run(x)
    return out
